# revision 18
# baseline (speedup 1.0000x reference)
"""nn_Block_SpeGroup — Bass/Tile kernel for 8 NeuronCores (data-parallel over B).

Layout strategy (per core, NB=4 samples):
  - in_proj/conv/silu in channel-major [ch, token] via PE transpose + matmul,
    staged to DRAM, re-read as scan tiles xs_k [hr, L] (interval packing done
    by strided DMA + DVE free-permutes for the k=1,3 packing and k>=2 flips).
  - selective scan: partition = (n_sub, hr) 128 lanes, free = L=1024; the
    recurrence runs on the DVE tensor_tensor_scan instruction. B/C/delta
    replication across partitions is folded into PE matmuls (expanded weights
    host-side). exp(delta*A) is one ACT op (per-partition scale = A).
  - y = sum_n h*C via PE matmul with a constant summing matrix, accumulated
    over 4 n-groups in PSUM.
  - inverse interval mapping via DVE free-permutes + DRAM bounce, epilogue
    (SE-gate, LayerNorm, z-gate, out_proj) in channel'-major where channel
    order is host-permuted so the interleave never materializes on device.
"""
import numpy as np

B, H, W, DIM = 32, 32, 32, 128
K, N, R = 4, 16, 2
L = 1024
EPS = 1e-5
NB = 4
NCORES = 8

PERM = np.array([4 * (c % 32) + c // 32 for c in range(128)], np.int64)

LAST_HW_EXEC_NS = None
_CACHE = {}
_MAX_WAITS = 1


def _host_prep(inp):
    f32 = np.float32
    in_proj_w = np.asarray(inp["in_proj_w"], f32)
    x_proj_weight = np.asarray(inp["x_proj_weight"], f32)
    dt_projs_weight = np.asarray(inp["dt_projs_weight"], f32)
    dt_projs_bias = np.asarray(inp["dt_projs_bias"], f32)
    A_logs = np.asarray(inp["A_logs"], f32)
    Ds = np.asarray(inp["Ds"], f32)

    c = {}
    c["W1Txx"] = np.ascontiguousarray(in_proj_w[:128, :].T)
    c["W1Tz"] = np.ascontiguousarray(in_proj_w[128 + PERM, :].T)
    c["convw"] = np.asarray(inp["conv_w"], f32).reshape(128, 1)
    c["convb"] = np.asarray(inp["conv_b"], f32).reshape(128, 1)

    MkT = np.zeros((32, 128), f32)
    for k in range(K):
        Mk = dt_projs_weight[k] @ x_proj_weight[k][:R]
        MkT[:, k * 32:(k + 1) * 32] = Mk.T
    c["MkT2"] = np.tile(MkT, (2, 1))
    c["dtb2"] = np.tile(np.ascontiguousarray(dt_projs_bias.T), (2, 1))

    p = np.arange(128)
    nsub, hr = p // 32, p % 32
    WbE = np.zeros((32, 16 * 128), f32)
    WcE = np.zeros((32, 16 * 128), f32)
    Acol = np.zeros((128, 16), f32)
    As = -np.exp(A_logs).reshape(K, 32, N)
    for k in range(K):
        for ng in range(4):
            n = 4 * ng + nsub
            WbE[:, (k * 4 + ng) * 128 + p] = x_proj_weight[k][R + n, :].T
            WcE[:, (k * 4 + ng) * 128 + p] = x_proj_weight[k][R + N + n, :].T
            Acol[:, k * 4 + ng] = As[k, hr, n]
    c["WbE2"] = np.tile(WbE, (2, 1))
    c["WcE2"] = np.tile(WcE, (2, 1))
    c["Acol"] = Acol

    c["Dcol2"] = np.tile(np.ascontiguousarray(Ds.reshape(K, 32).T), (2, 1))
    REP = np.zeros((32, 128), f32)
    REP[hr, p] = 1.0
    c["REP2"] = np.tile(REP, (2, 1))
    c["SUM4"] = np.ascontiguousarray(REP.T)
    c["onesM"] = np.full((128, 128), 1.0 / 128, f32)

    c["fc1Ts"] = np.ascontiguousarray(np.asarray(inp["fc1_w"], f32).T) / 1024.0
    c["fc1b"] = np.asarray(inp["fc1_b"], f32).reshape(4, 1)
    c["fc2TP"] = np.ascontiguousarray(np.asarray(inp["fc2_w"], f32)[PERM, :].T)
    c["fc2b"] = np.asarray(inp["fc2_b"], f32)[PERM].reshape(128, 1)
    c["lng"] = np.asarray(inp["ln_g"], f32)[PERM].reshape(128, 1)
    c["lnb"] = np.asarray(inp["ln_b"], f32)[PERM].reshape(128, 1)
    c["WoP"] = np.ascontiguousarray(
        np.asarray(inp["out_proj_w"], f32)[:, PERM].T)
    c["epscol"] = np.full((128, 1), EPS, f32)
    c["one64"] = np.ones((64, 1), f32)
    return c


_CONST_SHAPES = {
    "W1Txx": [128, 128], "W1Tz": [128, 128], "convw": [128, 1],
    "convb": [128, 1], "MkT2": [64, 128], "dtb2": [64, 4],
    "WbE2": [64, 2048], "WcE2": [64, 2048], "Acol": [128, 16],
    "Dcol2": [64, 4], "REP2": [64, 128], "SUM4": [128, 32],
    "onesM": [128, 128], "fc1Ts": [128, 4],
    "fc1b": [4, 1], "fc2TP": [4, 128], "fc2b": [128, 1],
    "lng": [128, 1], "lnb": [128, 1], "WoP": [128, 128],
    "epscol": [128, 1], "one64": [64, 1],
}


def _build_nc(act_native=True):
    import concourse.bass as bass
    import concourse.mybir as mybir
    from concourse.tile import TileContext
    from concourse.vector_clock import ScopedClock
    from concourse.masks import make_identity

    F32 = mybir.dt.float32
    AF = mybir.ActivationFunctionType
    OP = mybir.AluOpType
    AX = mybir.AxisListType

    class PatchedTileContext(TileContext):
        # This walrus build rejects >1 sync-wait on the final Drain; move the
        # extra waits onto dedicated nops right after it (SP executes in
        # order, so the following barrier still sees every wait satisfied).
        def _drain_and_barrier(self, tick_clock, wait_clock):
            drain_inst = self.nc.sync.drain()
            wait_clock.add_sem_waits(
                drain_inst.ins, ScopedClock({None: tick_clock.global_clock})
            )
            si = drain_inst.ins.sync_info
            waits = list(si.on_wait) if (si is not None and si.on_wait) else []
            if len(waits) > 1:
                si.on_wait = waits[:1]
                for w in waits[1:]:
                    n = self.nc.sync.nop(nofuse=True, hint="split_drain_wait")
                    nsi = n.ins.sync_info
                    if nsi is None:
                        n.ins.sync_info = mybir.SyncInfo(on_wait=[w], on_update=[])
                    else:
                        nsi.on_wait = [w]
            self.nc.all_engine_barrier()
            assert self.sems is not None
            popped = self.nc._tile_sem_poison_stack.pop()
            assert popped is self._sem_poison
            self.nc.clear_and_free_semaphores(list(self.sems.allocated().values()))
            self.nc.all_engine_barrier()

    nc = bass.Bass(target_bir_lowering=False)
    x4 = nc.dram_tensor("x4", [NB, 1024, 128], F32, kind="ExternalInput")
    cst = {n: nc.dram_tensor(n, s, F32, kind="ExternalInput")
           for n, s in _CONST_SHAPES.items()}
    out4 = nc.dram_tensor("out4", [NB, 1024, 128], F32, kind="ExternalOutput")
    xcstage = nc.dram_tensor("xcstage", [NB, 128, 1024], F32)
    ybounce = nc.dram_tensor("ybounce", [K, NB * 32, 1024], F32)

    from contextlib import ExitStack
    with PatchedTileContext(nc) as tc, ExitStack() as stack:
        consts = stack.enter_context(tc.tile_pool(name="consts", bufs=1))
        C = {}
        for n, s in _CONST_SHAPES.items():
            C[n] = consts.tile(s, F32, tag="c_" + n, name="c_" + n)
            nc.gpsimd.dma_start(out=C[n][:, :], in_=cst[n][:, :])
        idt = consts.tile([128, 128], F32, tag="idt")
        make_identity(nc, idt)

        persist = stack.enter_context(tc.tile_pool(name="persist", bufs=1))
        z_all = persist.tile([128, NB * 1024], F32, tag="z_all")
        f2_all = persist.tile([128, NB], F32, tag="f2_all")
        zz_all = persist.tile([128, NB], F32, tag="zz_all")
        yyP = persist.tile([128, NB * 1024], F32, tag="yyP")

        mid = stack.enter_context(tc.tile_pool(name="mid", bufs=1))
        # k-paired tiles: rows [0:32]=even k, [32:64]=odd k; b along free dim
        xsP = [mid.tile([64, NB * L], F32, tag=f"xsP{i}", name=f"xsP{i}")
               for i in range(2)]
        dlP = [mid.tile([64, NB * L], F32, tag=f"dlP{i}", name=f"dlP{i}")
               for i in range(2)]

        def krow(k):
            return xsP[k // 2], dlP[k // 2], slice(32 * (k % 2), 32 * (k % 2) + 32)

        # ---------------- P0: in_proj + conv/silu + z + SE-gate --------------
        # ACT ops are batched by function to avoid activation-table reloads.
        with tc.tile_pool(name="p0", bufs=2) as p0, \
             tc.tile_pool(name="p0b", bufs=1) as p0b, \
             tc.tile_pool(name="p0ps", bufs=2, space="PSUM") as p0ps:
            xcpre = [p0b.tile([128, 1024], F32, tag=f"xcpre{b}",
                              name=f"xcpre{b}") for b in range(NB)]
            xcsig = [p0b.tile([128, 1024], F32, tag=f"xcsig{b}",
                              name=f"xcsig{b}") for b in range(NB)]
            zsig = p0b.tile([128, NB * 1024], F32, tag="zsig")
            for b in range(NB):
                xTb = p0.tile([128, 1024], F32, tag="xTb")
                for cchunk in range(8):
                    xt = p0.tile([128, 128], F32, tag="xt")
                    nc.gpsimd.dma_start(
                        out=xt[:, :],
                        in_=x4[b, cchunk * 128:(cchunk + 1) * 128, :])
                    pt = p0ps.tile([128, 128], F32, tag="pt")
                    nc.tensor.transpose(pt[:, :], xt[:, :], idt[:, :])
                    nc.vector.tensor_copy(
                        xTb[:, cchunk * 128:(cchunk + 1) * 128], pt[:, :])
                for half in range(2):
                    wt = C["W1Txx"] if half == 0 else C["W1Tz"]
                    for ch2 in range(2):
                        pxz = p0ps.tile([128, 512], F32, tag="pxz")
                        nc.tensor.matmul(
                            pxz[:, :], wt[:, :],
                            xTb[:, ch2 * 512:(ch2 + 1) * 512],
                            start=True, stop=True)
                        sl = slice(ch2 * 512, (ch2 + 1) * 512)
                        if half == 0:
                            nc.scalar.activation(
                                xcpre[b][:, sl], pxz[:, :], AF.Identity,
                                bias=C["convb"][:, :], scale=C["convw"][:, :])
                        else:
                            nc.scalar.activation(
                                z_all[:, b * 1024 + ch2 * 512:
                                      b * 1024 + (ch2 + 1) * 512],
                                pxz[:, :], AF.Identity)
            for b in range(NB):
                nc.scalar.activation(xcsig[b][:, :], xcpre[b][:, :],
                                     AF.Sigmoid)
            nc.scalar.activation(zsig[:, :], z_all[:, :], AF.Sigmoid)
            for b in range(NB):
                xc_b = p0.tile([128, 1024], F32, tag="xc_b")
                nc.vector.tensor_mul(xc_b[:, :], xcpre[b][:, :],
                                     xcsig[b][:, :])
                nc.gpsimd.dma_start(out=xcstage[b, :, :], in_=xc_b[:, :])
                nc.vector.reduce_sum(zz_all[:, b:b + 1], xc_b[:, :], axis=AX.X)
            nc.vector.tensor_mul(z_all[:, :], z_all[:, :], zsig[:, :])
            pf1 = p0ps.tile([4, NB], F32, tag="pf1")
            nc.tensor.matmul(pf1[:, :], C["fc1Ts"][:, :], zz_all[:, :],
                             start=True, stop=True)
            f1 = p0.tile([4, NB], F32, tag="f1")
            nc.scalar.activation(f1[:, :], pf1[:, :], AF.Relu,
                                 bias=C["fc1b"][:, :])
            pf2 = p0ps.tile([128, NB], F32, tag="pf2")
            nc.tensor.matmul(pf2[:, :], C["fc2TP"][:, :], f1[:, :],
                             start=True, stop=True)
            nc.scalar.activation(f2_all[:, :], pf2[:, :], AF.Sigmoid,
                                 bias=C["fc2b"][:, :])

        # ---------------- P1: interval packing loads --------------------
        with tc.tile_pool(name="p1", bufs=1) as p1:
            rawP = [p1.tile([64, NB * L], F32, tag=f"rawP{i}", name=f"rawP{i}")
                    for i in range(2)]
            for b in range(NB):
                for k in range(K):
                    xst, _, rs = krow(k)
                    dst = xst if k == 0 else rawP[k // 2]
                    eng = nc.sync if (b + k) % 2 == 0 else nc.scalar
                    eng.dma_start(
                        out=dst[rs, b * L:(b + 1) * L]
                        .rearrange("p (j w) -> p j w", j=32),
                        in_=xcstage[b, k::4, :]
                        .rearrange("j (h w) -> h j w", h=32))
            # k=1: (j,w)->(w,j); k=2: block reverse; k=3: both
            nc.vector.tensor_copy(
                xsP[0][32:64].rearrange("p (b w j) -> p b w j", b=NB, w=32),
                rawP[0][32:64].rearrange("p (b j w) -> p b w j", b=NB, j=32))
            nc.vector.tensor_copy(
                xsP[1][0:32].rearrange("p (b m) -> p b m", b=NB),
                rawP[1][0:32].rearrange("p (b m) -> p b m", b=NB)[:, :, ::-1])
            nc.vector.tensor_copy(
                xsP[1][32:64].rearrange("p (b w j) -> p b w j", b=NB, w=32),
                rawP[1][32:64].rearrange("p (b j w) -> p b w j", b=NB, j=32)
                [:, :, ::-1, ::-1])

        # ---------------- P2: delta ----------------
        with tc.tile_pool(name="p2", bufs=1) as p2, \
             tc.tile_pool(name="p2ps", bufs=2, space="PSUM") as p2ps:
            eP = [p2.tile([64, NB * L], F32, tag=f"eP{i}", name=f"eP{i}")
                  for i in range(2)]
            for k in range(K):
                xst, dlt, rs = krow(k)
                for ch in range(NB * L // 512):
                    sl = slice(ch * 512, (ch + 1) * 512)
                    pd = p2ps.tile([64, 512], F32, tag="pd")
                    nc.tensor.matmul(
                        pd[rs, :], C["MkT2"][rs, k * 32:(k + 1) * 32],
                        xst[rs, sl], start=True, stop=True)
                    if act_native:
                        nc.scalar.activation(
                            dlt[rs, sl], pd[rs, :], AF.Softplus,
                            bias=C["dtb2"][rs, k:k + 1])
                    else:
                        nc.scalar.activation(
                            eP[k // 2][rs, sl], pd[rs, :], AF.Exp,
                            bias=C["dtb2"][rs, k:k + 1])
            if not act_native:
                for k in range(K):
                    xst, dlt, rs = krow(k)
                    nc.scalar.activation(
                        dlt[rs, :], eP[k // 2][rs, :], AF.Ln,
                        bias=C["one64"][rs, :])

        # ---------------- P3: scan main loop + inverse interval -------------
        with tc.tile_pool(name="p3", bufs=2) as p3, \
             tc.tile_pool(name="p3ps", bufs=1, space="PSUM") as p3ps:
            for b in range(NB):
                bs = slice(b * L, (b + 1) * L)
                for k in range(K):
                    xst, dlt, rs = krow(k)
                    pdrep = p3ps.tile([128, L], F32, tag="pdrep")
                    pdxs = p3ps.tile([128, L], F32, tag="pdxs_y")
                    for ch2 in range(2):
                        sl = slice(b * L + ch2 * 512, b * L + (ch2 + 1) * 512)
                        dl = slice(ch2 * 512, (ch2 + 1) * 512)
                        nc.tensor.matmul(
                            pdrep[:, dl], C["REP2"][rs, :],
                            dlt[rs, sl], start=True, stop=True)
                        nc.tensor.matmul(
                            pdxs[:, dl], C["REP2"][rs, :],
                            xst[rs, sl], start=True, stop=True)
                    xsrep = p3.tile([128, L], F32, tag="xsrep")
                    nc.vector.tensor_copy(xsrep[:, :], pdxs[:, :])
                    durep = p3.tile([128, L], F32, tag="durep")
                    nc.vector.tensor_mul(durep[:, :], xsrep[:, :],
                                         pdrep[:, :])
                    py = p3ps.tile([64, L], F32, tag="pdxs_y")
                    for ng in range(4):
                        col = k * 4 + ng
                        pB = p3ps.tile([128, L], F32, tag="pB")
                        pC = p3ps.tile([128, L], F32, tag="pC")
                        for ch2 in range(2):
                            sl = slice(b * L + ch2 * 512,
                                       b * L + (ch2 + 1) * 512)
                            dl = slice(ch2 * 512, (ch2 + 1) * 512)
                            nc.tensor.matmul(
                                pB[:, dl],
                                C["WbE2"][rs, col * 128:(col + 1) * 128],
                                xst[rs, sl], start=True, stop=True)
                            nc.tensor.matmul(
                                pC[:, dl],
                                C["WcE2"][rs, col * 128:(col + 1) * 128],
                                xst[rs, sl], start=True, stop=True)
                        a_t = p3.tile([128, L], F32, tag="a_t")
                        nc.scalar.activation(a_t[:, :], pdrep[:, :], AF.Exp,
                                             scale=C["Acol"][:, col:col + 1])
                        bin_t = p3.tile([128, L], F32, tag="bin_t")
                        nc.vector.tensor_mul(bin_t[:, :], durep[:, :],
                                             pB[:, :])
                        h_t = p3.tile([128, L], F32, tag="h_t")
                        nc.vector.tensor_tensor_scan(
                            h_t[:, :], a_t[:, :], bin_t[:, :], 0.0,
                            OP.mult, OP.add)
                        hC_t = p3.tile([128, L], F32, tag="hC_t")
                        nc.vector.tensor_mul(hC_t[:, :], h_t[:, :], pC[:, :])
                        for ch2 in range(2):
                            dl = slice(ch2 * 512, (ch2 + 1) * 512)
                            nc.tensor.matmul(
                                py[rs, dl], C["SUM4"][:, :], hC_t[:, dl],
                                start=(ng == 0), stop=(ng == 3))
                    yb_t = p3.tile([64, L], F32, tag="yb_t")
                    nc.vector.tensor_scalar(
                        yb_t[rs, :], xst[rs, bs], C["Dcol2"][rs, k:k + 1],
                        None, OP.mult)
                    nc.vector.tensor_add(yb_t[rs, :], yb_t[rs, :], py[rs, :])
                    # stage A: free-permute back to token order, then bounce
                    yp_t = p3.tile([64, L], F32, tag="yp_t")
                    if k == 0:
                        nc.vector.tensor_copy(
                            yp_t[rs].rearrange("p (j w) -> p j w", j=32),
                            yb_t[rs].rearrange("p (w j) -> p j w", w=32))
                    elif k == 1:
                        nc.vector.tensor_copy(yp_t[rs, :], yb_t[rs, :])
                    elif k == 2:
                        nc.vector.tensor_copy(
                            yp_t[rs].rearrange("p (j w) -> p j w", j=32),
                            yb_t[rs].rearrange("p (w j) -> p j w", w=32)
                            [:, ::-1, ::-1])
                    else:
                        nc.vector.tensor_copy(yp_t[rs, :], yb_t[rs, ::-1])
                    nc.gpsimd.dma_start(
                        out=ybounce[k, 32 * b:32 * (b + 1), :],
                        in_=yp_t[rs, :])

        # ---------------- P5: gather yyP (one DMA per direction) -------------
        for k in range(K):
            eng = nc.sync if k % 2 == 0 else nc.scalar
            eng.dma_start(
                out=yyP[32 * k:32 * (k + 1), :]
                .rearrange("p (q w) -> p q w", w=32),
                in_=ybounce[k].rearrange("q (j w) -> j q w", j=32))

        # ---------------- P6: SE-mult, LayerNorm, z-gate, out_proj ----------
        with tc.tile_pool(name="p6", bufs=2) as p6, \
             tc.tile_pool(name="p6n", bufs=1) as p6n, \
             tc.tile_pool(name="p6ps", bufs=2, space="PSUM") as p6ps:
            yyg = p6n.tile([128, NB * 1024], F32, tag="yyg")
            for b in range(NB):
                nc.vector.tensor_scalar(
                    yyg[:, b * 1024:(b + 1) * 1024],
                    yyP[:, b * 1024:(b + 1) * 1024],
                    f2_all[:, b:b + 1], None, OP.mult)
            tsub = p6n.tile([128, NB * 1024], F32, tag="tsub")
            for ch in range(8):
                sl = slice(ch * 512, (ch + 1) * 512)
                pmu = p6ps.tile([128, 512], F32, tag="pmu")
                nc.tensor.matmul(pmu[:, :], C["onesM"][:, :], yyg[:, sl],
                                 start=True, stop=True)
                nc.vector.tensor_sub(tsub[:, sl], yyg[:, sl], pmu[:, :])
            sq = p6n.tile([128, NB * 1024], F32, tag="yyg")
            nc.scalar.activation(sq[:, :], tsub[:, :], AF.Square)
            std = p6n.tile([128, NB * 1024], F32, tag="std")
            for ch in range(8):
                sl = slice(ch * 512, (ch + 1) * 512)
                pvar = p6ps.tile([128, 512], F32, tag="pvar")
                nc.tensor.matmul(pvar[:, :], C["onesM"][:, :], sq[:, sl],
                                 start=True, stop=True)
                nc.scalar.activation(std[:, sl], pvar[:, :], AF.Sqrt,
                                     bias=C["epscol"][:, :])
            rinv = p6n.tile([128, NB * 1024], F32, tag="yyg")
            nc.vector.reciprocal(rinv[:, :], std[:, :])
            yyn = p6n.tile([128, NB * 1024], F32, tag="std")
            nc.vector.tensor_mul(yyn[:, :], tsub[:, :], rinv[:, :])
            yyf = p6n.tile([128, NB * 1024], F32, tag="yyf")
            nc.scalar.activation(yyf[:, :], yyn[:, :], AF.Identity,
                                 bias=C["lnb"][:, :], scale=C["lng"][:, :])
            nc.vector.tensor_mul(yyf[:, :], yyf[:, :], z_all[:, :])
            for chunk in range(32):
                b, cb = chunk // 8, chunk % 8
                po = p6ps.tile([128, 128], F32, tag="po")
                nc.tensor.matmul(
                    po[:, :],
                    yyf[:, chunk * 128:(chunk + 1) * 128],
                    C["WoP"][:, :], start=True, stop=True)
                ot = p6.tile([128, 128], F32, tag="ot")
                nc.vector.tensor_copy(ot[:, :], po[:, :])
                nc.sync.dma_start(
                    out=out4[b, cb * 128:(cb + 1) * 128, :], in_=ot[:, :])

    # This walrus build rejects instructions carrying more than one sync
    # wait. Move every extra wait onto a dedicated NoOp inserted just before
    # the instruction on the same engine (engines execute in order, so the
    # waits still gate the instruction).
    wsplit = [0]
    for fn in nc.m.functions:
        for blk in fn.blocks:
            newlist = []
            for inst in blk.instructions:
                si = getattr(inst, "sync_info", None)
                waits = list(si.on_wait) if (si is not None and si.on_wait) else []
                if len(waits) > _MAX_WAITS:
                    si.on_wait = waits[-_MAX_WAITS:]
                    extra = waits[:-_MAX_WAITS]
                    for i in range(0, len(extra), _MAX_WAITS):
                        wsplit[0] += 1
                        nop = mybir.InstNoOp(
                            name=f"I-wsplit-{wsplit[0]}", ins=[], outs=[])
                        nop.engine = inst.engine
                        nop.sync_info = mybir.SyncInfo(
                            on_wait=extra[i:i + _MAX_WAITS], on_update=[])
                        nc.register_instruction(nop)
                        newlist.append(nop)
                newlist.append(inst)
            blk.instructions[:] = newlist
    return nc


def _get_nc(act_native=True):
    key = ("nc", act_native)
    if key not in _CACHE:
        _CACHE[key] = _build_nc(act_native)
    return _CACHE[key]


def _make_runner(nc, n_cores):
    """Cache a jitted shard_map executable for nc plus a timing variant that
    executes the NEFF `_TIME_REPS` times back-to-back (the BassEffect on the
    custom call serializes them), so (t_reps - t_one) / (reps - 1) isolates
    device execution from dispatch/transfer overhead."""
    import jax
    from jax.sharding import Mesh, PartitionSpec
    try:
        from jax.experimental.shard_map import shard_map
    except Exception:
        from jax.shard_map import shard_map
    import concourse.mybir as mybir
    from concourse import bass2jax
    bass2jax.install_neuronx_cc_hook()

    partition_name = (nc.partition_id_tensor.name
                      if nc.partition_id_tensor else None)
    in_names, out_names, out_avals, zero_shapes = [], [], [], []
    for alloc in nc.m.functions[0].allocations:
        if not isinstance(alloc, mybir.MemoryLocationSet):
            continue
        name = alloc.memorylocations[0].name
        if alloc.kind == "ExternalInput":
            if name != partition_name:
                in_names.append(name)
        elif alloc.kind == "ExternalOutput":
            out_names.append(name)
            shape = tuple(alloc.tensor_shape)
            dtype = mybir.dt.np(alloc.dtype)
            out_avals.append(jax.core.ShapedArray(shape, dtype))
            zero_shapes.append((shape, dtype))
    n_params, n_outs = len(in_names), len(out_names)
    all_names = list(in_names) + list(out_names)
    if partition_name is not None:
        all_names.append(partition_name)
    all_names = tuple(all_names)
    donate = tuple(range(n_params, n_params + n_outs))

    # x4 and out4 share a shape: feeding rep i's output back as rep i+1's
    # input forms a data chain so XLA cannot CSE/DCE the repeated NEFF
    # executions in the timing variant.
    def body_reps(reps):
        def _body(*args):
            outs = None
            operands = list(args)
            if partition_name is not None:
                operands.append(bass2jax.partition_id_tensor())
            for _ in range(reps):
                outs = bass2jax._bass_exec_p.bind(
                    *operands, out_avals=tuple(out_avals),
                    in_names=all_names, out_names=tuple(out_names),
                    lowering_input_output_aliases=(),
                    sim_require_finite=True, sim_require_nnan=True, nc=nc)
                operands = [outs[0]] + operands[1:]
            return tuple(outs)
        return _body

    devices = jax.devices()[:n_cores]
    mesh = Mesh(np.asarray(devices), ("core",))
    specs_in = (PartitionSpec("core"),) * (n_params + n_outs)
    specs_out = (PartitionSpec("core"),) * n_outs

    def jit_of(reps):
        return jax.jit(
            shard_map(body_reps(reps), mesh=mesh, in_specs=specs_in,
                      out_specs=specs_out, check_rep=False),
            donate_argnums=donate, keep_unused=True)

    return dict(jit1=jit_of(1), jitN=jit_of(_TIME_REPS), in_names=in_names,
                n_params=n_params, zero_shapes=zero_shapes,
                out_names=out_names, out_avals=out_avals, n_cores=n_cores)


_TIME_REPS = 17


def _concat_inputs(runner, in_maps):
    n_cores = runner["n_cores"]
    per_core = [[np.asarray(m[nm]) for nm in runner["in_names"]]
                for m in in_maps]
    return [np.concatenate([per_core[c][i] for c in range(n_cores)], axis=0)
            for i in range(runner["n_params"])]


def _zeros(runner):
    n_cores = runner["n_cores"]
    return [np.zeros((n_cores * s[0], *s[1:]), d)
            for s, d in runner["zero_shapes"]]


def kernel(**inputs):
    global LAST_HW_EXEC_NS
    import jax, time

    c = _host_prep(inputs)
    x = np.asarray(inputs["x"], np.float32)
    # act_native=False: this walrus build's LowerAct pass rejects the
    # Silu/Softplus PWP tables, so they are decomposed (Sigmoid*x, Ln(Exp+1)).
    nc = _get_nc(act_native=False)
    if "runner" not in _CACHE:
        _CACHE["runner"] = _make_runner(nc, NCORES)
    runner = _CACHE["runner"]

    in_maps = []
    for i in range(NCORES):
        m = {"x4": np.ascontiguousarray(
            x[i * NB:(i + 1) * NB].reshape(NB, 1024, 128))}
        for n in _CONST_SHAPES:
            m[n] = c[n]
        in_maps.append(m)
    concat_in = _concat_inputs(runner, in_maps)

    out_arrs = jax.block_until_ready(runner["jit1"](*concat_in, *_zeros(runner)))
    outs = [np.asarray(a) for a in out_arrs]

    try:
        t1 = 1e30
        tN = 1e30
        for _ in range(2):
            t0 = time.perf_counter()
            jax.block_until_ready(runner["jit1"](*concat_in, *_zeros(runner)))
            t1 = min(t1, time.perf_counter() - t0)
        for _ in range(2):
            t0 = time.perf_counter()
            jax.block_until_ready(runner["jitN"](*concat_in, *_zeros(runner)))
            tN = min(tN, time.perf_counter() - t0)
        LAST_HW_EXEC_NS = max(int((tN - t1) / (_TIME_REPS - 1) * 1e9), 1)
    except Exception:
        pass

    out4 = outs[0].reshape(NCORES, NB, 1024, 128)
    out = out4.reshape(B, H, W, DIM)
    return out.astype(np.float32)
